# revision 47
# baseline (speedup 1.0000x reference)
"""Trainium2 Bass kernel for nn_AnchorDeformAtt (deformable anchor attention).

Sharding: spatial L-shard across 8 cores -- core i handles pixels
l in [512i, 512(i+1)) for BOTH batches and ALL heads. Zero collectives;
the host concatenates per-core output shards.

Design:
  - Memory: quad rows (m[j], m[j+1], m[j+64], m[j+65]) bf16, so ONE
    index per sample point fetches all 4 bilinear taps (d=4). 16
    ap_gathers per core (2 half-gathers x (b, hg, l-block), num_idxs
    =2048) -- minimal index count, which is what the backend charges.
  - Value conv in bf16 (feat staged bf16 host-side); bias folded out
    (softmax x bilinear weights sum to 1 => Wout @ bv is a constant
    output bias, merged with bn_beta host-side; BN scale folded into
    Wout). Prep convs in float32r (tf32 rate).
  - Gather stream per head 16-partition group: i = (p, lhi, lq) with
    row q = lq = l%16, cols (p, lhi). Indices are wrapped at STORE
    time into DRAM [k][hg][b][h][lq][p][lhi] (the strided store pays
    the transpose tax once); idxw loads are clean 2-dim DMAs.
  - u = attn * bilinear weights staged as [b][(h,p)][l][dx4],
    broadcast to each head's 32 channels with 3-dim-AP DMAs.
  - Combine matmuls (contraction over 4 heads x 32 ch, stride-4 rhs
    per (p, dx)) accumulate psum[o, 256l] over (hg, hf, p, dx); one
    ACT copy + DMA out per (b, lblk, oc).
  - Emission interleaves prep phases between gather blocks so the
    gather engine never starves; idx stores ride SP behind ft loads,
    idxw loads on ACT.
"""
from contextlib import ExitStack

import numpy as np
import ml_dtypes

import concourse.bass as bass
import concourse.mybir as mybir
import concourse.tile as tile
from concourse import bacc
from concourse.bass_utils import run_bass_kernel_spmd

NH, NP = 8, 16
B, C, H, W = 2, 256, 64, 64
L = H * W            # 4096
NCORES = 8
LSH = L // NCORES    # 512
LPAD = L + 64        # pairs-memory rows (y1 tap reads idx+64)
EPS = 1e-6
F32 = mybir.dt.float32
BF16 = mybir.dt.bfloat16
I16 = mybir.dt.int16
F32R = mybir.dt.float32r

_GRAPH_CACHE = {}

PARAM_SPECS = {
    'featb': ([128, B, 2, L], BF16),
    'fsh': ([128, B, 2, LSH], F32R),
    'wv_t': ([128, 2, 2, 128], BF16),    # K, hg, kc, M
    'woff_t': ([128, 2, 2, 128], F32R),   # K, xy, kc, M
    'boff_p': ([128, 2], F32),           # per-partition bias, xy
    'wsz_t': ([128, 2, 2, 8], F32R),      # K, xy, kc, 8
    'bsz_p': ([8, 2], F32),
    'sel8': ([8, 128], F32),             # sel8[h, h*16+p] = 1
    'watt_t': ([128, 2, 128], F32R),      # K, kc, N
    'batt_r': ([1, 128], F32),
    'ones1': ([1, 128], F32),
    'ident': ([128, 128], F32),
    'wout_t': ([128, 2, 2, 128], BF16),  # K, hg, oc, M (bn-scaled)
    'obias': ([128, 2], F32),            # (Wout_sc @ bv + beta) as [m, oc]
    'cen2': ([128, 2, LSH], F32),        # packed (x|y) centers
}


def build_graph(stub_gather=False):
    key = (stub_gather,)
    if key in _GRAPH_CACHE:
        return _GRAPH_CACHE[key]

    nc = bacc.Bacc("TRN2", target_bir_lowering=False, debug=False,
                   num_devices=NCORES)
    dp = nc.declare_dram_parameter
    P = {n: dp(n, s, dt, isOutput=False) for n, (s, dt) in PARAM_SPECS.items()}
    out_e = dp("out", [B, 2, 128, LSH], F32, isOutput=True)

    # wrapped y0 idx staging, one tensor per (k, hg): [b][h4][lq][p][lhi]
    idxd = {(k, hg): nc.dram_tensor(f"idxd{k}{hg}", [B, 4, 16, 16, 16], I16)
            for k in range(2) for hg in range(2)}
    # u staging: [b][(h,p) 128][tap 2][l 512][dx 2]
    ud = nc.dram_tensor("ud", [B, 128, 2, LSH, 2], BF16)

    AP = bass.AP
    Act = mybir.ActivationFunctionType
    Alu = mybir.AluOpType

    with tile.TileContext(nc) as tc, ExitStack() as ctx:
        consts = ctx.enter_context(tc.tile_pool(name="consts", bufs=1))
        featp = ctx.enter_context(tc.tile_pool(name="featp", bufs=2))
        fshp = ctx.enter_context(tc.tile_pool(name="fshp", bufs=1))
        memp = ctx.enter_context(tc.tile_pool(name="memp", bufs=1))
        prep = ctx.enter_context(tc.tile_pool(name="prep", bufs=1))
        gm = ctx.enter_context(tc.tile_pool(name="gm", bufs=1))
        idxwp = ctx.enter_context(tc.tile_pool(name="idxwp", bufs=2))
        ubcp = ctx.enter_context(tc.tile_pool(name="ubcp", bufs=2))
        gathp = ctx.enter_context(tc.tile_pool(name="gathp", bufs=3))
        outp = ctx.enter_context(tc.tile_pool(name="outp", bufs=2))
        ps_v = ctx.enter_context(tc.tile_pool(name="ps_v", bufs=2, space="PSUM"))
        ps_p = ctx.enter_context(tc.tile_pool(name="ps_p", bufs=2, space="PSUM"))
        ps_o = ctx.enter_context(tc.tile_pool(name="ps_o", bufs=1, space="PSUM"))

        def dmas(out, in_):          # SP queue: consts, ft, idx stores, ubc
            nc.sync.dma_start(out=out, in_=in_)

        def dmaa(out, in_):          # ACT queue: fsh, idxw loads, uq, out
            nc.scalar.dma_start(out=out, in_=in_)

        # ---------------- constants ----------------
        def cload(name):
            shape, dt = PARAM_SPECS[name]
            t = consts.tile(list(shape), dt, tag=name, name=f"c_{name}")
            dmas(t[:], P[name].ap())
            return t

        wv_sb = cload('wv_t')
        woff_sb = cload('woff_t')
        boff_sb = cload('boff_p')
        wsz_sb = cload('wsz_t')
        bsz_sb = cload('bsz_p')
        sel8_sb = cload('sel8')
        watt_sb = cload('watt_t')
        batt_sb = cload('batt_r')
        ones_sb = cload('ones1')
        id_sb = cload('ident')
        wout_sb = cload('wout_t')
        obias_sb = cload('obias')
        cen_sb = cload('cen2')

        mem_t = {}
        st = {}

        # ---------------- value conv (bf16 pairs, no bias) ----------------
        def conv_sec(b, hg):
            q = memp.tile([128, L, 4], BF16, tag="quad",
                          name=f"quad{b}{hg}", bufs=2)
            mem_t[(b, hg)] = q
            nc.vector.memset(q[:, L - 65:, :], 0.0)
            for c in range(4):
                ft = featp.tile([128, 2, 1024], BF16, tag="ft")
                dmas(ft[:], AP(tensor=P['featb'], offset=b * 2 * L + c * 1024,
                               ap=[[B * 2 * L, 128], [L, 2], [1, 1024]]))
                for j in range(2):
                    n = c * 2 + j
                    ps = ps_v.tile([128, 512], F32, tag="pv")
                    for kc in range(2):
                        nc.tensor.matmul(ps[:], wv_sb[:, hg, kc, :],
                                         ft[:, kc, j * 512:(j + 1) * 512],
                                         start=(kc == 0), stop=(kc == 1))
                    for dxi, sh in ((0, 0), (1, 1), (2, 64), (3, 65)):
                        if sh == 0:
                            o, i = q[:, n * 512:(n + 1) * 512, 0], ps[:]
                        elif n == 0:
                            o, i = q[:, 0:512 - sh, dxi], ps[:, sh:512]
                        else:
                            o = q[:, n * 512 - sh:(n + 1) * 512 - sh, dxi]
                            i = ps[:]
                        if dxi < 2:
                            nc.scalar.activation(out=o, in_=i, func=Act.Copy)
                        else:
                            nc.vector.tensor_copy(out=o, in_=i)

        # ---------------- prep phases (per b) ----------------
        def prep_a(b):
            """fsh load; offset+size convs -> packed offp/szbp [128,2,LSH]."""
            s = st.setdefault(b, {})
            fsh = fshp.tile([128, 2, LSH], F32R, tag="fsh")
            dmaa(fsh[:], P['fsh'].ap()[:, b, :, :])
            s['fsh'] = fsh
            offp = prep.tile([128, 2, LSH], F32, tag="offp")
            szbp = prep.tile([128, 2, LSH], F32, tag="szbp")
            s['offp'], s['szbp'] = offp, szbp
            for xy in range(2):
                ps = ps_p.tile([128, 512], F32, tag="pp", name="psz")
                for kc in range(2):
                    nc.tensor.matmul(ps[0:8, :], wsz_sb[:, xy, kc, :],
                                     fsh[:, kc, :], start=(kc == 0),
                                     stop=(kc == 1))
                szs = gm.tile([8, LSH], F32, tag="szs")
                nc.scalar.activation(out=szs[:], in_=ps[0:8, :],
                                     func=Act.Sigmoid,
                                     bias=bsz_sb[:, xy:xy + 1], scale=1.0)
                nc.vector.tensor_scalar(out=szs[:], in0=szs[:], scalar1=0.75,
                                        scalar2=0.25, op0=Alu.min, op1=Alu.max)
                psb = ps_p.tile([128, 512], F32, tag="pp", name="psb")
                nc.tensor.matmul(psb[:], sel8_sb[:], szs[:],
                                 start=True, stop=True)
                nc.vector.tensor_copy(out=szbp[:, xy, :], in_=psb[:])
                ps2 = ps_p.tile([128, 512], F32, tag="pp", name="po")
                for kc in range(2):
                    nc.tensor.matmul(ps2[:], woff_sb[:, xy, kc, :],
                                     fsh[:, kc, :], start=(kc == 0),
                                     stop=(kc == 1))
                nc.scalar.activation(out=offp[:, xy, :], in_=ps2[:],
                                     func=Act.Sigmoid,
                                     bias=boff_sb[:, xy:xy + 1], scale=1.0)

        def prep_c(b):
            """grid -> floor -> flat y0 idx (packed x|y in one [128,2,LSH])."""
            s = st[b]
            offp, szbp = s['offp'], s['szbp']
            o2 = offp[:].rearrange("p a b -> p (a b)")
            s2 = szbp[:].rearrange("p a b -> p (a b)")
            cf = gm.tile([128, 2, LSH], F32, tag="cf")
            c2 = cf[:].rearrange("p a b -> p (a b)")
            ci = gm.tile([128, 2, LSH], I16, tag="ci")
            i2 = ci[:].rearrange("p a b -> p (a b)")
            msk = gm.tile([128, 2, LSH], F32, tag="msk")
            m2 = msk[:].rearrange("p a b -> p (a b)")
            nc.vector.tensor_scalar(out=o2, in0=o2, scalar1=-0.5,
                                    scalar2=None, op0=Alu.add)
            nc.vector.tensor_tensor(out=o2, in0=o2, in1=s2, op=Alu.mult)
            nc.vector.tensor_tensor(
                out=o2, in0=o2,
                in1=cen_sb[:].rearrange("p a b -> p (a b)"), op=Alu.add)
            nc.vector.tensor_scalar(out=o2, in0=o2, scalar1=1.0, scalar2=0.0,
                                    op0=Alu.min, op1=Alu.max)
            nc.vector.tensor_scalar(out=o2, in0=o2, scalar1=float(W - 1),
                                    scalar2=None, op0=Alu.mult)
            nc.vector.tensor_copy(out=i2, in_=o2)
            nc.vector.tensor_copy(out=c2, in_=i2)
            nc.vector.tensor_tensor(out=m2, in0=c2, in1=o2, op=Alu.is_gt)
            nc.vector.tensor_tensor(out=c2, in0=c2, in1=m2, op=Alu.subtract)
            nc.vector.tensor_tensor(out=o2, in0=o2, in1=c2, op=Alu.subtract)
            # flat y0 = y0f*W + x0f -> reuse szbp x-half as scratch, fi -> ci
            fl = szbp[:, 0, :]
            nc.vector.tensor_scalar(out=fl, in0=cf[:, 1, :],
                                    scalar1=float(W), scalar2=None,
                                    op0=Alu.mult)
            nc.vector.tensor_tensor(out=fl, in0=fl, in1=cf[:, 0, :],
                                    op=Alu.add)
            fi = gm.tile([128, LSH], I16, tag="fi")
            nc.vector.tensor_copy(out=fi[:], in_=fl)
            s['fi'] = fi
            # wx/wy in offp halves; cf/msk slots free for prep_d reuse
            s['cf'], s['msk'] = cf, msk

        def prep_cs(b, k):
            """Wrapped y0 idx stores for l-block k + clean idxw loads +
            DVE-derived y1 idx tiles."""
            s = st[b]
            fi = s['fi']
            for hg in range(2):
                for hh in range(4):
                    h = hg * 4 + hh
                    dmas(AP(tensor=idxd[(k, hg)], offset=(b * 4 + hh) * 4096,
                            ap=[[16, 16], [1, 16], [256, 16]]),
                         fi[h * 16:(h + 1) * 16, k * 256:(k + 1) * 256])
                ix = idxwp.tile([128, 256], I16, tag=f"ix{hg}{k}",
                                name=f"ix{b}{hg}{k}")
                st[('ix', b, hg, k)] = ix
                for hh in range(4):
                    for dup in range(2):
                        r = hh * 32 + dup * 16
                        dmaa(ix[r:r + 16, :],
                             AP(tensor=idxd[(k, hg)],
                                offset=(b * 4 + hh) * 4096,
                                ap=[[256, 16], [1, 256]]))


        def prep_b(b):
            """attn conv (pixel-major) + softmax + transpose -> aT."""
            s = st[b]
            fsh = s['fsh']
            aT = prep.tile([128, LSH], F32, tag="aT")
            s['aT'] = aT
            for lb in range(LSH // 128):
                ps = ps_p.tile([128, 128], F32, tag="pp", name="pa")
                for kc in range(2):
                    nc.tensor.matmul(ps[:], fsh[:, kc, lb * 128:(lb + 1) * 128],
                                     watt_sb[:, kc, :], start=(kc == 0),
                                     stop=False)
                nc.tensor.matmul(ps[:], ones_sb[:], batt_sb[:],
                                 start=False, stop=True)
                ae = gm.tile([128, 8, 16], F32, tag="ae")
                nc.scalar.activation(out=ae[:], in_=ps[:], func=Act.Exp)
                ssum = gm.tile([128, 8, 1], F32, tag="ssum")
                nc.vector.tensor_reduce(out=ssum[:], in_=ae[:],
                                        axis=mybir.AxisListType.X, op=Alu.add)
                nc.vector.reciprocal(out=ssum[:], in_=ssum[:])
                for h in range(NH):
                    nc.vector.tensor_scalar(out=ae[:, h, :], in0=ae[:, h, :],
                                            scalar1=ssum[:, h, :],
                                            scalar2=None, op0=Alu.mult)
                pst = ps_p.tile([128, 128], F32, tag="pp", name="pt")
                nc.tensor.transpose(pst[:], ae[:].rearrange("p a b -> p (a b)"),
                                    id_sb[:])
                nc.scalar.activation(out=aT[:, lb * 128:(lb + 1) * 128],
                                     in_=pst[:], func=Act.Copy)

        def prep_d(b):
            """u = attn * bilinear -> upair [128, tap, l, dx] -> DRAM."""
            s = st[b]
            offp, cf, msk, aT = s['offp'], s['cf'], s['msk'], s['aT']
            wx, wy = offp[:, 0, :], offp[:, 1, :]
            omx, omy = msk[:, 0, :], msk[:, 1, :]
            ay0, ay1 = cf[:, 0, :], cf[:, 1, :]
            nc.vector.tensor_scalar(out=omx, in0=wx, scalar1=-1.0,
                                    scalar2=1.0, op0=Alu.mult, op1=Alu.add)
            nc.vector.tensor_scalar(out=omy, in0=wy, scalar1=-1.0,
                                    scalar2=1.0, op0=Alu.mult, op1=Alu.add)
            nc.vector.tensor_tensor(out=ay0, in0=aT[:], in1=omy, op=Alu.mult)
            nc.vector.tensor_tensor(out=ay1, in0=aT[:], in1=wy, op=Alu.mult)
            uq = gm.tile([128, LSH, 4], BF16, tag="uq")
            for dxi, (yf, xf) in enumerate(((ay0, omx), (ay0, wx),
                                            (ay1, omx), (ay1, wx))):
                nc.vector.tensor_tensor(out=uq[:, :, dxi], in0=yf,
                                        in1=xf, op=Alu.mult)
            dmaa(AP(tensor=ud, offset=b * 128 * 2048,
                    ap=[[2048, 128], [1, 2048]]),
                 uq[:].rearrange("p a b -> p (a b)"))

        # ---------------- gather + combine ----------------
        pso = {}
        cnt = {}

        def gblock(b, hg, k):
            quad = mem_t[(b, hg)]
            idxw = st[('ix', b, hg, k)]
            for oc in range(2):
                if (b, k, oc) not in pso:
                    pso[(b, k, oc)] = ps_o.tile([128, 256], F32,
                                                tag=f"po{k}{oc}",
                                                name=f"po{b}{k}{oc}")
                    cnt[(b, k, oc)] = 0
            for hf in range(2):       # half-gathers: p in [8hf, 8hf+8)
                g = gathp.tile([128, 2048, 4], BF16, tag="g4")
                if stub_gather:
                    nc.gpsimd.ap_gather(
                        g[:, 0:16, :], quad[:].rearrange("p a b -> p (a b)"),
                        idxw[:, hf * 128:hf * 128 + 1], channels=128,
                        num_elems=L, d=4, num_idxs=16)
                else:
                    nc.gpsimd.ap_gather(
                        g[:], quad[:].rearrange("p a b -> p (a b)"),
                        idxw[:, hf * 128:(hf + 1) * 128], channels=128,
                        num_elems=L, d=4, num_idxs=2048)
                ubc = ubcp.tile([128, 8192], BF16, tag="ubc")
                for hh in range(4):
                    # split the broadcast traffic across both DMA queues
                    dmaq = dmas if hh % 2 == 0 else dmaa
                    dmaq(ubc[hh * 32:(hh + 1) * 32, :],
                         AP(tensor=ud,
                            offset=(b * 128 + (hg * 4 + hh) * 16) * 2048
                            + hf * 8 * 2048 + k * 1024,
                            ap=[[0, 32], [2048, 8], [1, 1024]]))
                nc.vector.tensor_tensor(
                    out=g[:].rearrange("p a b -> p (a b)"),
                    in0=g[:].rearrange("p a b -> p (a b)"),
                    in1=ubc[:], op=Alu.mult)
                gap = g[:]
                for oc in range(2):
                    for p in range(8):
                        for dxi in range(4):
                            rhs = AP(tensor=gap.tensor,
                                     offset=gap.offset + p * 1024 + dxi,
                                     ap=[gap.ap[0], [4, 256]])
                            c = cnt[(b, k, oc)]
                            nc.tensor.matmul(
                                pso[(b, k, oc)][:],
                                wout_sb[:, hg, oc, :], rhs,
                                start=(c == 0), stop=(c == 127))
                            cnt[(b, k, oc)] = c + 1

        def finalize(b, k):
            for oc in range(2):
                o_sb = outp.tile([128, 256], F32, tag="osb")
                nc.scalar.activation(out=o_sb[:], in_=pso[(b, k, oc)][:],
                                     func=Act.Identity,
                                     bias=obias_sb[:, oc:oc + 1], scale=1.0)
                dmaa(AP(tensor=out_e,
                        offset=((b * 2 + oc) * 128) * LSH + k * 256,
                        ap=[[LSH, 128], [1, 256]]), o_sb[:])

        # ---------------- emission schedule ----------------
        prep_a(0)
        prep_c(0)
        conv_sec(0, 0)
        conv_sec(0, 1)
        prep_cs(0, 0)
        prep_b(0)
        prep_d(0)
        gblock(0, 0, 0)
        prep_cs(0, 1)
        gblock(0, 0, 1)
        prep_a(1)
        prep_b(1)
        conv_sec(1, 0)
        gblock(0, 1, 0)
        finalize(0, 0)
        prep_c(1)
        prep_cs(1, 0)
        gblock(0, 1, 1)
        finalize(0, 1)
        prep_d(1)
        prep_cs(1, 1)
        conv_sec(1, 1)
        gblock(1, 0, 0)
        gblock(1, 0, 1)
        gblock(1, 1, 0)
        finalize(1, 0)
        gblock(1, 1, 1)
        finalize(1, 1)

    nc.compile()
    _GRAPH_CACHE[key] = nc
    return nc


def stage_inputs(inputs, core):
    """Build the per-core in_map (all arrays pre-laid-out for plain DMAs)."""
    bf16 = ml_dtypes.bfloat16
    feat = np.ascontiguousarray(
        np.asarray(inputs['feat_sd'], np.float32).reshape(B, C, L))
    lo = core * LSH
    WvT = np.asarray(inputs['value_proj_w'], np.float32).T.copy()
    WoffT = np.asarray(inputs['anchor_deform_w'], np.float32).T.copy()
    WattT = np.asarray(inputs['anchor_att_w'], np.float32).T.copy()
    WszT = np.asarray(inputs['size_deform_w'], np.float32).T.copy()
    WoutT = np.asarray(inputs['out_proj_w'], np.float32).T.copy()
    boff = np.asarray(inputs['anchor_deform_b'], np.float32)
    bsz = np.asarray(inputs['size_deform_b'], np.float32)
    bv = np.asarray(inputs['value_proj_b'], np.float32)
    bn_s = (np.asarray(inputs['bn_gamma'], np.float32)
            / np.sqrt(np.float32(1.0 + 1e-5)))
    beta = np.asarray(inputs['bn_beta'], np.float32)
    WoutT_sc = WoutT * bn_s[None, :]
    obias = (bv @ WoutT_sc + beta).reshape(2, 128).T
    sel8 = np.zeros((8, 128), np.float32)
    for h in range(8):
        sel8[h, h * 16:(h + 1) * 16] = 1.0
    cols = (np.arange(W) + 0.5) / (W + EPS)
    rows = (np.arange(H) + 0.5) / (H + EPS)
    cx = np.tile(cols, H)[lo:lo + LSH].astype(np.float32)
    cy = np.repeat(rows, W)[lo:lo + LSH].astype(np.float32)
    cen2 = np.stack([np.broadcast_to(cx, (128, LSH)),
                     np.broadcast_to(cy, (128, LSH))], axis=1)
    # woff/wsz packed: xy-interleaved output channels split into x|y planes
    woff = np.stack([WoffT[:, 0::2], WoffT[:, 1::2]],
                    axis=1)                      # [256, 2, 128]
    woff_t = woff.reshape(2, 128, 2, 128).transpose(1, 2, 0, 3)
    wsz = np.stack([WszT[:, 0::2], WszT[:, 1::2]], axis=1)  # [256, 2, 8]
    wsz_t = wsz.reshape(2, 128, 2, 8).transpose(1, 2, 0, 3)
    fr = feat.reshape(B, 2, 128, L)
    m = {
        'featb': np.ascontiguousarray(
            fr.transpose(2, 0, 1, 3)).astype(bf16),
        'fsh': np.ascontiguousarray(
            fr[:, :, :, lo:lo + LSH].transpose(2, 0, 1, 3)),
        'wv_t': np.ascontiguousarray(
            WvT.reshape(2, 128, 2, 128).transpose(1, 2, 0, 3)).astype(bf16),
        'woff_t': np.ascontiguousarray(woff_t),
        'boff_p': np.ascontiguousarray(
            np.stack([boff[0::2], boff[1::2]], axis=1)),
        'wsz_t': np.ascontiguousarray(wsz_t),
        'bsz_p': np.ascontiguousarray(
            np.stack([bsz[0::2], bsz[1::2]], axis=1)),
        'sel8': sel8,
        'watt_t': np.ascontiguousarray(
            WattT.reshape(2, 128, 128).transpose(1, 0, 2)),
        'batt_r': np.asarray(inputs['anchor_att_b'],
                             np.float32).reshape(1, 128),
        'ones1': np.ones((1, 128), np.float32),
        'ident': np.eye(128, dtype=np.float32),
        'wout_t': np.ascontiguousarray(
            WoutT_sc.reshape(2, 128, 2, 128).transpose(1, 0, 2, 3)
        ).astype(bf16),
        'obias': np.ascontiguousarray(obias),
        'cen2': np.ascontiguousarray(cen2),
    }
    return m


def kernel(**inputs):
    nc = build_graph()
    in_maps = [stage_inputs(inputs, i) for i in range(NCORES)]
    res = run_bass_kernel_spmd(nc, in_maps, core_ids=list(range(NCORES)))
    shards = [res.results[i]['out'].reshape(B, C, LSH) for i in range(NCORES)]
    full = np.concatenate(shards, axis=2).reshape(B, C, H, W)
    return full.astype(np.float32)
